# revision 1
# baseline (speedup 1.0000x reference)
"""FP4Linear on 8 TRN2 NeuronCores.

Computes out[B,S,Do] = x[B,S,Di] @ (codes[Do,Di] * s).T + bias[Do].

Sharding: tokens 4-way x out_features 2-way (each core gets a disjoint
[2048 tok, 2048 of] output block; x row-shards and W row-shards are
replicated across the matching axis). This halves per-core HBM reads vs
pure column-parallel (x would be fully replicated).

Per-core kernel (Tile framework):
  - W shard is shipped as fp16 (int4 codes are exactly representable),
    transposed on load HBM->SBUF via xbar DMA-transpose into
    [128k, 32kb, 512of] resident tiles.
  - x tile [128tok, 4096] is cast fp32->fp16 during the SWDGE DMA, then
    xbar-transposed SBUF->SBUF into [128k, 32kb, 128tok].
  - 32 fp16 matmuls accumulate per PSUM bank [128tok, 512of] (fp32).
  - Eviction: ScalarE copy with per-partition scale AP (= weight_scale),
    then one VectorE add of the partition-broadcast bias; output leaves
    via SWDGE so the HWDGE ring belongs to the xbar transposes alone.

All xbar transposes stay on the single nc.sync HWDGE queue: issuing them
concurrently from both HWDGE queues (sync + scalar) corrupts data on HW
(cross-queue xbar race).
"""

import sys

import numpy as np

if "/opt/trn_rl_repo" not in sys.path:
    sys.path.insert(0, "/opt/trn_rl_repo")

import concourse.mybir as mybir  # noqa: E402
import concourse.tile as tile  # noqa: E402
from concourse import bacc  # noqa: E402
from concourse.bass_utils import run_bass_kernel_spmd  # noqa: E402

P = 128
MM_N = 512  # psum bank free dim (fp32)

N_CORES = 8
TOK_SHARDS = 4
OF_SHARDS = 2


def build_nc(
    tok: int, d_in: int, of: int, reps: int = 1, dyn_reps: int = 1,
    accum_out: bool = False,
):
    """One core's program: out[tok, of] = x[tok, d_in] @ w[of, d_in].T * s + b.

    reps>1 repeats the whole token loop statically; dyn_reps>1 wraps it in a
    hardware For_i loop. Both recompute the same output and exist only for
    steady-state timing measurements (per-dispatch overhead cancels).
    """
    kb_n = d_in // P  # k blocks
    tt_n = tok // P  # token tiles
    nof = of // MM_N  # psum chunks along out features

    nc = bacc.Bacc("TRN2", target_bir_lowering=False)
    x_d = nc.dram_tensor("x", [tok, d_in], mybir.dt.float32, kind="ExternalInput")
    w_d = nc.dram_tensor("w", [of, d_in], mybir.dt.float16, kind="ExternalInput")
    b_d = nc.dram_tensor("b", [of], mybir.dt.float32, kind="ExternalInput")
    s_d = nc.dram_tensor("s", [1], mybir.dt.float32, kind="ExternalInput")
    o_d = nc.dram_tensor("o", [tok, of], mybir.dt.float32, kind="ExternalOutput")

    hwdge = [nc.sync, nc.sync]  # single HWDGE queue (cross-queue xbar races?)

    with tile.TileContext(nc) as tc:
        with (
            tc.tile_pool(name="const", bufs=1) as cpool,
            tc.tile_pool(name="wt", bufs=1) as wtpool,
            tc.tile_pool(name="xin", bufs=2) as xpool,
            tc.tile_pool(name="xt", bufs=4) as xtpool,
            tc.tile_pool(name="out", bufs=2) as opool,
            tc.tile_pool(name="ps", bufs=8, space="PSUM") as pspool,
        ):
            # Resident transposed weights: nof tensors [P, kb_n, MM_N] fp16.
            # Each is filled by two xbar transposes (split along k) spread
            # over both HWDGE queues so the first matmuls can start sooner.
            kb_half = kb_n // 2

            def emit_wt(c, wt_c, splits=2):
                kb_piece = kb_n // splits
                for h in range(splits):
                    hwdge[h % 2].dma_start_transpose(
                        wt_c[:, h * kb_piece : (h + 1) * kb_piece, :],
                        w_d[
                            c * MM_N : (c + 1) * MM_N,
                            h * kb_piece * P : (h + 1) * kb_piece * P,
                        ],
                    )

            def emit_x(t):
                x_nat = xpool.tile([P, d_in], mybir.dt.float16, tag="xnat")
                # SWDGE DMA casts fp32 -> fp16 in flight.
                nc.gpsimd.dma_start(x_nat[:], x_d[t * P : (t + 1) * P, :])
                xt_t = xtpool.tile([P, kb_n, P], mybir.dt.float16, tag="xt")
                hwdge[t % 2].dma_start_transpose(xt_t[:], x_nat[:])
                return xt_t

            # Emission order shapes per-queue HW order: first of-chunk's
            # weights, then the first x tiles, then the remaining weights —
            # so tile 0's matmul chunks become runnable incrementally.
            wts = [
                wtpool.tile(
                    [P, kb_n, MM_N], mybir.dt.float16, tag=f"wt{c}", name=f"wt{c}"
                )
                for c in range(nof)
            ]
            # wt0 in quarters: the first matmul chain starts after ~1/4 of
            # wt0's bytes instead of 1/2 (deps are AP-range granular).
            emit_wt(0, wts[0], splits=4)
            prefetched = {t: emit_x(t) for t in range(min(4, tt_n))}

            # Constants are first needed at the first eviction — emit after
            # the x prefetch so their SWDGE descriptor generation (the
            # stride-0 bias broadcast is slow on the Q7) doesn't delay it.
            s_t = cpool.tile([P, 1], mybir.dt.float32, tag="s")
            nc.gpsimd.dma_start(s_t[:], s_d[None, :].to_broadcast((P, 1)))
            bias_t = cpool.tile([P, of], mybir.dt.float32, tag="bias")
            nc.gpsimd.dma_start(bias_t[:], b_d[None, :].to_broadcast((P, of)))

            for c in range(1, nof):
                emit_wt(c, wts[c])

            def token_pass(use_prefetch):
                for t in range(tt_n):
                    if use_prefetch and t in prefetched:
                        xt_t = prefetched.pop(t)
                    else:
                        xt_t = emit_x(t)

                    o_t = opool.tile([P, of], mybir.dt.float32, tag="o", name="o_t")
                    for c in range(nof):
                        ps = pspool.tile(
                            [P, MM_N], mybir.dt.float32, tag="ps", name="ps"
                        )
                        for kb in range(kb_n):
                            nc.tensor.matmul(
                                ps[:],
                                xt_t[:, kb, :],
                                wts[c][:, kb, :],
                                start=(kb == 0),
                                stop=(kb == kb_n - 1),
                            )
                        # out = psum * s  (ACT copy, per-partition scale AP)
                        nc.scalar.mul(
                            o_t[:, c * MM_N : (c + 1) * MM_N], ps[:], s_t[:, 0:1]
                        )
                    # out += bias (broadcast along partitions)
                    nc.vector.tensor_add(o_t[:], o_t[:], bias_t[:])
                    if accum_out:
                        # timing/validation builds: count loop trips in the sum
                        nc.gpsimd.dma_start(
                            o_d[t * P : (t + 1) * P, :],
                            o_t[:],
                            accum_op=mybir.AluOpType.add,
                        )
                    else:
                        # Output stores ride the second HWDGE ring (ACT):
                        # the SP ring stays transpose-only, SWDGE stays
                        # x-load-only. (Plain SBUF->DRAM copies on the other
                        # HWDGE ring don't hit the cross-queue xbar race —
                        # that pairing was transpose-vs-transpose.)
                        nc.scalar.dma_start(o_d[t * P : (t + 1) * P, :], o_t[:])

            if dyn_reps > 1:
                with tc.For_i(0, dyn_reps, 1):
                    token_pass(False)
            else:
                for rep in range(reps):
                    token_pass(rep == 0)

    nc.compile()
    return nc


_NC_CACHE: dict = {}


def _get_nc(tok: int, d_in: int, of: int):
    key = (tok, d_in, of)
    if key not in _NC_CACHE:
        _NC_CACHE[key] = build_nc(tok, d_in, of)
    return _NC_CACHE[key]


def make_in_maps(x, fp4_weight, weight_scale, bias):
    """Shard full inputs into 8 per-core input maps."""
    b, s, d_in = x.shape
    d_out = fp4_weight.shape[0]
    tok = (b * s) // TOK_SHARDS
    of = d_out // OF_SHARDS

    xf = np.ascontiguousarray(np.asarray(x, dtype=np.float32).reshape(b * s, d_in))
    w16 = np.ascontiguousarray(np.asarray(fp4_weight).astype(np.float16))
    b32 = np.ascontiguousarray(np.asarray(bias, dtype=np.float32))
    s32 = np.ascontiguousarray(np.asarray(weight_scale, dtype=np.float32).reshape(1))

    in_maps = []
    for core in range(N_CORES):
        ti, oi = divmod(core, OF_SHARDS)
        in_maps.append(
            {
                "x": xf[ti * tok : (ti + 1) * tok],
                "w": w16[oi * of : (oi + 1) * of],
                "b": b32[oi * of : (oi + 1) * of],
                "s": s32,
            }
        )
    return in_maps, (b, s, d_in, d_out, tok, of)


def kernel(x, fp4_weight, weight_scale, bias, **run_kwargs):
    in_maps, (b, s, d_in, d_out, tok, of) = make_in_maps(
        x, fp4_weight, weight_scale, bias
    )
    nc = _get_nc(tok, d_in, of)
    res = run_bass_kernel_spmd(nc, in_maps, core_ids=list(range(N_CORES)), **run_kwargs)

    out = np.empty((b * s, d_out), dtype=np.float32)
    for core in range(N_CORES):
        ti, oi = divmod(core, OF_SHARDS)
        out[ti * tok : (ti + 1) * tok, oi * of : (oi + 1) * of] = res.results[core]["o"]
    out = out.reshape(b, s, d_out)
    if run_kwargs:
        return out, res
    return out



# revision 2
# speedup vs baseline: 1.3008x; 1.3008x over previous
"""FP4Linear on 8 TRN2 NeuronCores.

Computes out[B,S,Do] = x[B,S,Di] @ (codes[Do,Di] * s).T + bias[Do].

Sharding: tokens 4-way x out_features 2-way (each core gets a disjoint
[2048 tok, 2048 of] output block; x row-shards and W row-shards are
replicated across the matching axis). This halves per-core HBM reads vs
pure column-parallel (x would be fully replicated).

Per-core kernel (Tile framework):
  - W shard is shipped already transposed+packed on the host as fp16
    (int4 codes are exactly representable): DRAM layout [128 kpart,
    nof, kb_n*512] so resident SBUF tiles [128, kb_n, 512] fill via
    straight contiguous DMA (no xbar transposes for W at all). Chunk 0
    is loaded in 4 k-slices so the first matmuls unblock early.
  - x tile [128tok, 4096] is cast fp32->fp16 during the SWDGE DMA, then
    xbar-transposed SBUF->SBUF into [128k, 32kb, 128tok] on the sync
    HWDGE ring (which carries nothing else). Tile 0 is loaded and
    transposed in 4 k-quarters to minimize the startup window.
  - 32 fp16 matmuls accumulate per PSUM bank [128tok, 512of] (fp32).
  - Eviction per 512-of chunk: ScalarE copy with per-partition scale AP
    (= weight_scale), VectorE bias add, store via the scalar HWDGE ring.
"""

import sys

import numpy as np

if "/opt/trn_rl_repo" not in sys.path:
    sys.path.insert(0, "/opt/trn_rl_repo")

import concourse.mybir as mybir  # noqa: E402
import concourse.tile as tile  # noqa: E402
from concourse import bacc  # noqa: E402
from concourse.bass_utils import run_bass_kernel_spmd  # noqa: E402

P = 128
MM_N = 512  # psum bank free dim (fp32)

N_CORES = 8
TOK_SHARDS = 4
OF_SHARDS = 2


def build_nc(tok: int, d_in: int, of: int):
    """One core's program: out[tok, of] = x[tok, d_in] @ w[of, d_in].T * s + b."""
    kb_n = d_in // P  # k blocks
    tt_n = tok // P  # token tiles
    nof = of // MM_N  # psum chunks along out features

    nc = bacc.Bacc("TRN2", target_bir_lowering=False)
    x_d = nc.dram_tensor("x", [tok, d_in], mybir.dt.float32, kind="ExternalInput")
    # pre-transposed on host: w[p, c, kb*512 + of_rel] = W[c*512+of_rel, kb*128+p]
    w_d = nc.dram_tensor(
        "w", [P, nof, kb_n * MM_N], mybir.dt.float16, kind="ExternalInput"
    )
    b_d = nc.dram_tensor("b", [of], mybir.dt.float32, kind="ExternalInput")
    s_d = nc.dram_tensor("s", [1], mybir.dt.float32, kind="ExternalInput")
    o_d = nc.dram_tensor("o", [tok, of], mybir.dt.float32, kind="ExternalOutput")

    with tile.TileContext(nc) as tc:
        with (
            tc.tile_pool(name="const", bufs=1) as cpool,
            tc.tile_pool(name="wt", bufs=1) as wtpool,
            tc.tile_pool(name="xin", bufs=2) as xpool,
            tc.tile_pool(name="xt", bufs=4) as xtpool,
            tc.tile_pool(name="out", bufs=6) as opool,
            tc.tile_pool(name="ps", bufs=8, space="PSUM") as pspool,
        ):
            wts = [
                wtpool.tile(
                    [P, kb_n, MM_N], mybir.dt.float16, tag=f"wt{c}", name=f"wt{c}"
                )
                for c in range(nof)
            ]

            def emit_x(t, splits=1):
                # SWDGE DMA casts fp32 -> fp16 in flight; sync-ring xbar
                # transpose into k-major. k-split loads/transposes unblock
                # the first matmuls of tile t after 1/splits of its bytes.
                x_nat = xpool.tile([P, d_in], mybir.dt.float16, tag="xnat")
                xt_t = xtpool.tile([P, kb_n, P], mybir.dt.float16, tag="xt")
                kq = d_in // splits
                kbq = kb_n // splits
                for q in range(splits):
                    nc.gpsimd.dma_start(
                        x_nat[:, q * kq : (q + 1) * kq],
                        x_d[t * P : (t + 1) * P, q * kq : (q + 1) * kq],
                    )
                    nc.sync.dma_start_transpose(
                        xt_t[:, q * kbq : (q + 1) * kbq, :],
                        x_nat[:, q * kq : (q + 1) * kq],
                    )
                return xt_t

            # x tile 0 in quarters: first matmul gate is ~1/4 of a tile.
            prefetched = {0: emit_x(0, splits=4)}

            # W chunk 0 in quarters on the scalar HWDGE ring (deps are
            # AP-range granular, so MMs of kb block j wait only their slice).
            for piece in range(4):
                kslc = slice(piece * (kb_n // 4) * MM_N, (piece + 1) * (kb_n // 4) * MM_N)
                nc.scalar.dma_start(wts[0][:, piece * (kb_n // 4) : (piece + 1) * (kb_n // 4), :], w_d[:, 0, kslc])

            for t in (1, 2, 3):
                prefetched[t] = emit_x(t)

            # Constants after the x prefetch: their SWDGE descriptor gen
            # (stride-0 broadcast is slow on the Q7) must not delay x.
            s_t = cpool.tile([P, 1], mybir.dt.float32, tag="s")
            nc.gpsimd.dma_start(s_t[:], s_d[None, :].to_broadcast((P, 1)))
            bias_t = cpool.tile([P, of], mybir.dt.float32, tag="bias")
            nc.gpsimd.dma_start(bias_t[:], b_d[None, :].to_broadcast((P, of)))

            for c in range(1, nof):
                nc.scalar.dma_start(wts[c][:], w_d[:, c, :])

            for t in range(tt_n):
                xt_t = prefetched.pop(t) if t in prefetched else emit_x(t)

                for c in range(nof):
                    ps = pspool.tile([P, MM_N], mybir.dt.float32, tag="ps", name="ps")
                    for kb in range(kb_n):
                        nc.tensor.matmul(
                            ps[:],
                            xt_t[:, kb, :],
                            wts[c][:, kb, :],
                            start=(kb == 0),
                            stop=(kb == kb_n - 1),
                        )
                    # out = psum * s  (ACT copy, per-partition scale AP)
                    o_t = opool.tile([P, MM_N], mybir.dt.float32, tag="o", name="o_t")
                    nc.scalar.mul(o_t[:], ps[:], s_t[:, 0:1])
                    # out += bias (partition-broadcast), then store
                    nc.vector.tensor_add(
                        o_t[:], o_t[:], bias_t[:, c * MM_N : (c + 1) * MM_N]
                    )
                    nc.scalar.dma_start(
                        o_d[t * P : (t + 1) * P, c * MM_N : (c + 1) * MM_N], o_t[:]
                    )

    nc.compile()
    return nc


_NC_CACHE: dict = {}


def _get_nc(tok: int, d_in: int, of: int):
    key = (tok, d_in, of)
    if key not in _NC_CACHE:
        _NC_CACHE[key] = build_nc(tok, d_in, of)
    return _NC_CACHE[key]


def make_in_maps(x, fp4_weight, weight_scale, bias):
    """Shard full inputs into 8 per-core input maps."""
    b, s, d_in = x.shape
    d_out = fp4_weight.shape[0]
    tok = (b * s) // TOK_SHARDS
    of = d_out // OF_SHARDS
    nof = of // MM_N
    kb_n = d_in // P

    xf = np.ascontiguousarray(np.asarray(x, dtype=np.float32).reshape(b * s, d_in))
    w16 = np.asarray(fp4_weight).astype(np.float16)
    b32 = np.ascontiguousarray(np.asarray(bias, dtype=np.float32))
    s32 = np.ascontiguousarray(np.asarray(weight_scale, dtype=np.float32).reshape(1))

    in_maps = []
    for core in range(N_CORES):
        ti, oi = divmod(core, OF_SHARDS)
        wsh = w16[oi * of : (oi + 1) * of]  # [of, d_in]
        # [c, of_rel, kb, p] -> [p, c, kb*512+of_rel]
        wt = wsh.reshape(nof, MM_N, kb_n, P).transpose(3, 0, 2, 1)
        wt = np.ascontiguousarray(wt.reshape(P, nof, kb_n * MM_N))
        in_maps.append(
            {
                "x": xf[ti * tok : (ti + 1) * tok],
                "w": wt,
                "b": b32[oi * of : (oi + 1) * of],
                "s": s32,
            }
        )
    return in_maps, (b, s, d_in, d_out, tok, of)


def kernel(x, fp4_weight, weight_scale, bias, **run_kwargs):
    in_maps, (b, s, d_in, d_out, tok, of) = make_in_maps(
        x, fp4_weight, weight_scale, bias
    )
    nc = _get_nc(tok, d_in, of)
    res = run_bass_kernel_spmd(nc, in_maps, core_ids=list(range(N_CORES)), **run_kwargs)

    out = np.empty((b * s, d_out), dtype=np.float32)
    for core in range(N_CORES):
        ti, oi = divmod(core, OF_SHARDS)
        out[ti * tok : (ti + 1) * tok, oi * of : (oi + 1) * of] = res.results[core]["o"]
    out = out.reshape(b, s, d_out)
    if run_kwargs:
        return out, res
    return out
